# revision 7
# baseline (speedup 1.0000x reference)
"""Trainium2 Bass kernel for AxialAttention (attention along W axis).

Sharding: pure data-parallel over (B=4) x (H split in 2) = 8 shards, one
per NeuronCore. Attention mixes pixels only along W within a single
(b, head, h-row), so splitting H requires no collectives.

Per-core pipeline (shard = [C=512, 48 rows x 96 cols]):
  1. q/k projection GEMM: w_qk^T-as-lhsT, x-as-rhs -> qk [o, pix] (bf16)
  2. per row r (48):
     - v projection for the row: x-as-lhsT -> v [96 pix, 512] (seq-major)
     - scores^T = k^T.T @ q^T per head -> [j, i] in PSUM
     - expS = exp(0.125 * scores^T) via ACT -> bf16 SBUF (no max
       subtraction: |scores*scale| <~ 7 for randn-scaled inputs)
     - AV^T: lhsT=v[j,d], rhs=expS[j,i] -> out^T [d, i] per head, and
       column sums via ones-lhsT matmul (both into one PSUM bank pair)
     - recip of sums (DVE approx), normalize-multiply -> attn_out
       [(head,d), pix] bf16
  3. out projection GEMM + bias via ACT Identity -> y [512, pix] fp32
"""

import numpy as np
import ml_dtypes

import concourse.bass as bass
import concourse.tile as tile
from concourse import mybir

BF16 = mybir.dt.bfloat16
F32 = mybir.dt.float32

B, C, H, W = 4, 512, 96, 96
HEADS, D = 8, 64
SCALE = 0.125
NCORES = 8
RPC = H // 2          # 48 rows per core
PIX = RPC * W         # 4608 pixels per core
NPT = PIX // 512      # 9 pixel tiles of 512


def build_nc(apply_waitfix=True, stage=5):
    nc = bass.Bass(trn_type="TRN2")
    x_d = nc.declare_dram_parameter("x", [4, 128, PIX], BF16, isOutput=False)
    wqk_d = nc.declare_dram_parameter("wqk", [4, 128, 1024], BF16, isOutput=False)
    wv_d = nc.declare_dram_parameter("wv", [4, 128, 512], BF16, isOutput=False)
    wo_d = nc.declare_dram_parameter("wo", [4, 128, 512], BF16, isOutput=False)
    bias_d = nc.declare_dram_parameter("bias", [4, 128, 1], F32, isOutput=False)
    y_d = nc.declare_dram_parameter("y", [512, PIX], F32, isOutput=True)

    with tile.TileContext(nc) as tc:
        with (
            tc.tile_pool(name="persist", bufs=1) as persist,
            tc.tile_pool(name="vrow", bufs=4) as vrow,
            tc.tile_pool(name="attn", bufs=4) as attn,
            tc.tile_pool(name="ostage", bufs=3) as ostage,
            tc.tile_pool(name="ps", bufs=4, space="PSUM") as ps,
        ):
            # --- persistent loads -------------------------------------
            x_t, wqk_t, wv_t, wo_t, bias_t = [], [], [], [], []
            for cc in range(4):
                xt = persist.tile([128, PIX], BF16, tag=f"x{cc}")
                nc.sync.dma_start(out=xt[:, :], in_=x_d[cc])
                x_t.append(xt)
                wt = persist.tile([128, 1024], BF16, tag=f"wqk{cc}")
                nc.sync.dma_start(out=wt[:, :], in_=wqk_d[cc])
                wqk_t.append(wt)
                vt = persist.tile([128, 512], BF16, tag=f"wv{cc}")
                nc.sync.dma_start(out=vt[:, :], in_=wv_d[cc])
                wv_t.append(vt)
                ot = persist.tile([128, 512], BF16, tag=f"wo{cc}")
                nc.sync.dma_start(out=ot[:, :], in_=wo_d[cc])
                wo_t.append(ot)
                bt = persist.tile([128, 1], F32, tag=f"bias{cc}")
                nc.sync.dma_start(out=bt[:, :], in_=bias_d[cc])
                bias_t.append(bt)
            ones_t = persist.tile([96, 64], BF16, tag="ones")
            nc.vector.memset(ones_t[:, :], 1.0)

            attn_out = persist.tile([128, 4 * PIX], BF16, tag="attn_out")
            attn_out_c = attn_out.rearrange("p (c n) -> p c n", c=4)

            # --- phase 1: q/k projection ------------------------------
            import os
            qk_t = []
            for oc in range(8):
                qt = persist.tile([128, PIX], BF16, tag=f"qk{oc}")
                qk_t.append(qt)
                for pt in range(NPT):
                    qps = ps.tile([128, 1024], F32, tag="ps")
                    sl = slice(pt * 512, (pt + 1) * 512)
                    for cc in range(4):
                        nc.tensor.matmul(
                            qps[:, 0:512],
                            lhsT=wqk_t[cc][:, oc * 128:(oc + 1) * 128],
                            rhs=x_t[cc][:, sl],
                            start=(cc == 0), stop=(cc == 3),
                        )
                    nc.vector.tensor_copy(out=qt[:, sl], in_=qps[:, 0:512])

            # --- phase 2+3: per-row v projection + attention ----------
            for r in range(RPC if stage >= 2 else 0):
                rsl = slice(r * 96, (r + 1) * 96)
                # v for this row: [96 pix, 512 (head,d)]
                vps = ps.tile([128, 1024], F32, tag="ps")
                for cc in range(4):
                    nc.tensor.matmul(
                        vps[0:96, 0:512],
                        lhsT=x_t[cc][:, rsl],
                        rhs=wv_t[cc][:, :],
                        start=(cc == 0), stop=(cc == 3),
                    )
                v_sb = vrow.tile([96, 512], BF16)
                nc.vector.tensor_copy(out=v_sb[:, :], in_=vps[0:96, 0:512])

                if stage < 3:
                    continue
                # scores^T per head: [j, i] blocks, 4 heads per PSUM bank
                sps = ps.tile([128, 1024], F32, tag="ps")
                # concurrent row-group (K-offset 0 vs 64) matmuls must
                # target different PSUM banks: head h -> bank h%2
                for h in range(8):
                    qc, half = h // 2, 64 * (h % 2)
                    col = 512 * (h % 2) + 96 * (h // 2)
                    nc.tensor.matmul(
                        sps[0:96, col:col + 96],
                        lhsT=qk_t[4 + qc][half:half + 64, rsl],
                        rhs=qk_t[qc][half:half + 64, rsl],
                    )
                expS = attn.tile([96, 768], BF16)
                for k in range(2):
                    nc.scalar.activation(
                        out=expS[:, k * 384:(k + 1) * 384],
                        in_=sps[0:96, k * 512:k * 512 + 384],
                        func=mybir.ActivationFunctionType.Exp,
                        scale=SCALE,
                    )

                if stage < 4:
                    continue
                # AV^T (cols 0:384) and sums (cols 512:896)
                aps = ps.tile([128, 1024], F32, tag="ps")
                for h in range(8):
                    half, blk = 64 * (h % 2), 96 * (h // 2)
                    esl = expS[:, 384 * (h % 2) + 96 * (h // 2):
                               384 * (h % 2) + 96 * (h // 2) + 96]
                    nc.tensor.matmul(
                        aps[half:half + 64, blk:blk + 96],
                        lhsT=v_sb[:, h * 64:(h + 1) * 64],
                        rhs=esl,
                    )
                    nc.tensor.matmul(
                        aps[half:half + 64, 512 + blk:512 + blk + 96],
                        lhsT=ones_t[:, :],
                        rhs=esl,
                    )
                recip = attn.tile([128, 384], F32)
                nc.vector.reciprocal(out=recip[:, :], in_=aps[:, 512:896])
                nc.vector.tensor_tensor(
                    out=attn_out_c[:, :, rsl],
                    in0=aps[:, 0:384].rearrange("p (c i) -> p c i", c=4),
                    in1=recip.rearrange("p (c i) -> p c i", c=4),
                    op=mybir.AluOpType.mult,
                )

            # --- phase 4: out projection + bias -----------------------
            for oc in range(4 if stage >= 5 else 0):
                for pt in range(NPT):
                    sl = slice(pt * 512, (pt + 1) * 512)
                    ops_ = ps.tile([128, 1024], F32, tag="ps")
                    for cc in range(4):
                        nc.tensor.matmul(
                            ops_[:, 0:512],
                            lhsT=wo_t[cc][:, oc * 128:(oc + 1) * 128],
                            rhs=attn_out_c[:, cc, sl],
                            start=(cc == 0), stop=(cc == 3),
                        )
                    o_sb = ostage.tile([128, 512], F32)
                    nc.scalar.add(out=o_sb[:, :], in_=ops_[:, 0:512],
                                  add=bias_t[oc][:, :])
                    nc.sync.dma_start(
                        out=y_d[oc * 128:(oc + 1) * 128, sl], in_=o_sb[:, :])

    if apply_waitfix:
        split_excess_waits(nc)
    return nc


def shard_inputs(x, w_qkv, w_out, b_out):
    """Full inputs -> list of 8 per-core input maps."""
    bf = ml_dtypes.bfloat16
    wqk = np.ascontiguousarray(w_qkv[:1024].T).astype(bf).reshape(4, 128, 1024)
    wv = np.ascontiguousarray(w_qkv[1024:].T).astype(bf).reshape(4, 128, 512)
    wo = np.ascontiguousarray(w_out.T).astype(bf).reshape(4, 128, 512)
    bias = b_out.astype(np.float32).reshape(4, 128, 1)
    xb = x.astype(bf)  # [4, 512, 96, 96]
    in_maps = []
    for core in range(NCORES):
        b, half = core // 2, core % 2
        xs = np.ascontiguousarray(
            xb[b, :, half * RPC:(half + 1) * RPC, :]).reshape(4, 128, PIX)
        in_maps.append({"x": xs, "wqk": wqk, "wv": wv, "wo": wo, "bias": bias})
    return in_maps


def unshard_outputs(results):
    out = np.empty((B, C, H, W), np.float32)
    for core in range(NCORES):
        b, half = core // 2, core % 2
        out[b, :, half * RPC:(half + 1) * RPC, :] = (
            results[core]["y"].reshape(C, RPC, W))
    return out


# --- walrus workaround -------------------------------------------------
# The walrus build in this container rejects instructions carrying more
# than a small number of semaphore waits (1 for CTRL-queue NoOp/Drain).
# TileContext's exit drain can exceed that. Split: keep at most one wait
# on the original instruction and insert same-engine NoOps immediately
# before it, each carrying one of the excess waits.
def split_excess_waits(nc):
    import bass_rust
    n_split = 0
    for f in nc.m.functions:
        for blk in f.blocks:
            newlist = []
            changed = False
            for inst in blk.instructions:
                si = inst.sync_info
                w = list(si.on_wait) if si is not None else []
                if len(w) > 1:
                    *pre, last = w
                    for ci, wait in enumerate(pre):
                        nop = mybir.InstNoOp(
                            name=f"{inst.name}-wsplit{ci}", ins=[], outs=[])
                        nop.engine = inst.engine
                        nop.sync_info = bass_rust.SyncInfo(
                            on_update=[], on_wait=[wait])
                        newlist.append(nop)
                    inst.sync_info.on_wait = [last]
                    changed = True
                    n_split += 1
                newlist.append(inst)
            if changed:
                blk.instructions = newlist
    return n_split


_NC_CACHE = None


def kernel(x, w_qkv, w_out, b_out):
    global _NC_CACHE
    from concourse.bass_utils import run_bass_kernel_spmd
    if _NC_CACHE is None:
        _NC_CACHE = build_nc()
    in_maps = shard_inputs(x, w_qkv, w_out, b_out)
    res = run_bass_kernel_spmd(_NC_CACHE, in_maps, list(range(NCORES)))
    return unshard_outputs(res.results)


# revision 25
# speedup vs baseline: 285.3271x; 285.3271x over previous
"""Trainium2 Bass kernel for AxialAttention (attention along W axis).

Sharding: pure data-parallel over (B=4) x (H split in 2) = 8 shards, one
per NeuronCore. Attention mixes pixels only along W within a single
(b, head, h-row), so splitting H requires no collectives.

Per-core pipeline (shard = [C=512, 48 rows x 96 cols], pixels tiled in
12 groups of 384 = exactly 4 attention rows, so all phases pipeline):
  for each pixel-group t (4 rows):
    1. q/k projection GEMM for group t: w_qk^T-as-lhsT, x-as-rhs
    2. per row r in group: v projection (x-as-lhsT, seq-major out),
       scores^T = k^T.T @ q^T per head (row-group pairs -> separate
       PSUM banks), expS = exp(0.125*scores^T) on ACT (no max
       subtraction: |scores*scale| < 7 for these inputs), AV^T +
       column-sums matmuls, reciprocal + normalize-multiply -> attn_out
    3. out projection GEMM for group t + bias via ACT Identity -> y
"""

import numpy as np
import ml_dtypes

import concourse.bass as bass
import concourse.tile as tile
from concourse import mybir

BF16 = mybir.dt.bfloat16
F32 = mybir.dt.float32

B, C, H, W = 4, 512, 96, 96
HEADS, D = 8, 64
SCALE = 0.125
NCORES = 8
RPC = H // 2          # 48 rows per core
PIX = RPC * W         # 4608 pixels per core
GRP = 12              # pixel groups
GPIX = PIX // GRP     # 384 pixels per group = 4 rows


def build_nc(apply_waitfix=True):
    nc = bass.Bass(trn_type="TRN2")
    x_d = nc.declare_dram_parameter("x", [4, 128, PIX], BF16, isOutput=False)
    wqk_d = nc.declare_dram_parameter("wqk", [4, 128, 1024], BF16, isOutput=False)
    wv_d = nc.declare_dram_parameter("wv", [4, 128, 512], BF16, isOutput=False)
    wo_d = nc.declare_dram_parameter("wo", [4, 128, 512], BF16, isOutput=False)
    bias_d = nc.declare_dram_parameter("bias", [4, 128, 1], F32, isOutput=False)
    y_d = nc.declare_dram_parameter("y", [512, PIX], F32, isOutput=True)

    with tile.TileContext(nc) as tc:
        with (
            tc.tile_pool(name="persist", bufs=1) as persist,
            tc.tile_pool(name="vrow", bufs=6) as vrow,
            tc.tile_pool(name="attn", bufs=6) as attn,
            tc.tile_pool(name="ostage", bufs=4) as ostage,
            tc.tile_pool(name="psA", bufs=4, space="PSUM") as psA,
            tc.tile_pool(name="psB", bufs=2, space="PSUM") as psB,
        ):
            # --- persistent loads (wqk + first x tiles first so the
            # projection GEMMs start as early as possible) --------------
            wqk_t, wv_t, wo_t, bias_t = [], [], [], []
            for cc in range(4):
                wt = persist.tile([128, 1024], BF16, tag=f"wqk{cc}")
                nc.sync.dma_start(out=wt[:, :], in_=wqk_d[cc])
                wqk_t.append(wt)
            # x in growing chunks: early groups land fast, later ones
            # amortize the ~0.65us per-DMA sequencer dispatch cost
            CHUNKS = [(0, 1), (1, 2), (2, 4), (4, 6), (6, 9), (9, 12)]
            x_t = [[None] * GRP for _ in range(4)]
            for ci, (t0, t1) in enumerate(CHUNKS):
                w = (t1 - t0) * GPIX
                for cc in range(4):
                    xt = persist.tile([128, w], BF16, tag=f"x{cc}_c{ci}")
                    nc.sync.dma_start(
                        out=xt[:, :],
                        in_=x_d[cc][:, t0 * GPIX:t1 * GPIX])
                    for t in range(t0, t1):
                        x_t[cc][t] = xt[:, (t - t0) * GPIX:(t - t0 + 1) * GPIX]
                if ci == 0:
                    for cc in range(4):
                        vt = persist.tile([128, 512], BF16, tag=f"wv{cc}")
                        nc.sync.dma_start(out=vt[:, :], in_=wv_d[cc])
                        wv_t.append(vt)
                    ones_t = persist.tile([96, 64], BF16, tag="ones")
                    nc.vector.memset(ones_t[:, :], 1.0)
                elif ci == 1:
                    for cc in range(4):
                        ot = persist.tile([128, 512], BF16, tag=f"wo{cc}")
                        nc.sync.dma_start(out=ot[:, :], in_=wo_d[cc])
                        wo_t.append(ot)
                        bt = persist.tile([128, 1], F32, tag=f"bias{cc}")
                        nc.sync.dma_start(out=bt[:, :], in_=bias_d[cc])
                        bias_t.append(bt)

            qk_t = [[None] * GRP for _ in range(8)]
            attn_t = [None] * GRP

            def emit_qk(t):
                for oc in range(8):
                    qps = psA.tile([128, 512], F32, tag="psA")
                    for cc in range(4):
                        nc.tensor.matmul(
                            qps[:, 0:GPIX],
                            lhsT=wqk_t[cc][:, oc * 128:(oc + 1) * 128],
                            rhs=x_t[cc][t][:, :],
                            start=(cc == 0), stop=(cc == 3),
                        )
                    qt = persist.tile([128, GPIX], BF16, tag=f"qk{oc}_{t}")
                    nc.vector.tensor_copy(out=qt[:, :], in_=qps[:, 0:GPIX])
                    qk_t[oc][t] = qt

            def emit_row_front(t, rr):
                """v projection + scores + exp for row rr of group t."""
                rsl = slice(rr * 96, rr * 96 + 96)
                vps = psA.tile([128, 512], F32, tag="psA")
                for cc in range(4):
                    nc.tensor.matmul(
                        vps[0:96, 0:512],
                        lhsT=x_t[cc][t][:, rsl],
                        rhs=wv_t[cc][:, :],
                        start=(cc == 0), stop=(cc == 3),
                    )
                v_sb = vrow.tile([96, 512], BF16)
                nc.vector.tensor_copy(out=v_sb[:, :], in_=vps[0:96, 0:512])

                # scores^T per head: [j, i]; concurrent row-group
                # (K-offset 0 vs 64) matmuls must hit different PSUM
                # banks: head h -> bank h%2
                sps = psB.tile([128, 1024], F32, tag="psB")
                for h in range(8):
                    qc, half = h // 2, 64 * (h % 2)
                    col = 512 * (h % 2) + 96 * (h // 2)
                    nc.tensor.matmul(
                        sps[0:96, col:col + 96],
                        lhsT=qk_t[4 + qc][t][half:half + 64, rsl],
                        rhs=qk_t[qc][t][half:half + 64, rsl],
                    )
                expS = attn.tile([96, 768], BF16)
                for k in range(2):
                    nc.scalar.activation(
                        out=expS[:, k * 384:(k + 1) * 384],
                        in_=sps[0:96, k * 512:k * 512 + 384],
                        func=mybir.ActivationFunctionType.Exp,
                        scale=SCALE,
                    )
                return v_sb, expS

            def emit_row_back(t, rr, v_sb, expS):
                """AV + sums matmuls, reciprocal, normalize for a row."""
                rsl = slice(rr * 96, rr * 96 + 96)
                at_c = attn_t[t].rearrange("p (c n) -> p c n", c=4)
                aps = psB.tile([128, 1024], F32, tag="psB")
                for h in range(8):
                    half, blk = 64 * (h % 2), 96 * (h // 2)
                    ecol = 384 * (h % 2) + 96 * (h // 2)
                    nc.tensor.matmul(
                        aps[half:half + 64, blk:blk + 96],
                        lhsT=v_sb[:, h * 64:(h + 1) * 64],
                        rhs=expS[:, ecol:ecol + 96],
                    )
                # column sums for all heads of one parity in one matmul
                # (expS is parity-major: cols 0:384 = even heads)
                for par in range(2):
                    nc.tensor.matmul(
                        aps[64 * par:64 * par + 64, 512:896],
                        lhsT=ones_t[:, :],
                        rhs=expS[:, 384 * par:384 * par + 384],
                    )
                recip = attn.tile([128, 384], F32)
                nc.vector.reciprocal(out=recip[:, :], in_=aps[:, 512:896])
                nc.vector.tensor_tensor(
                    out=at_c[:, :, rsl],
                    in0=aps[:, 0:384].rearrange("p (c i) -> p c i", c=4),
                    in1=recip.rearrange("p (c i) -> p c i", c=4),
                    op=mybir.AluOpType.mult,
                )

            def emit_outproj(t, ocs=range(4)):
                at_c = attn_t[t].rearrange("p (c n) -> p c n", c=4)
                for oc in ocs:
                    ops_ = psA.tile([128, 512], F32, tag="psA")
                    for cc in range(4):
                        nc.tensor.matmul(
                            ops_[:, 0:GPIX],
                            lhsT=wo_t[cc][:, oc * 128:(oc + 1) * 128],
                            rhs=at_c[:, cc, :],
                            start=(cc == 0), stop=(cc == 3),
                        )
                    o_sb = ostage.tile([128, GPIX], F32)
                    nc.scalar.add(out=o_sb[:, :], in_=ops_[:, 0:GPIX],
                                  add=bias_t[oc][:, :])
                    nc.gpsimd.dma_start(
                        out=y_d[oc * 128:(oc + 1) * 128,
                                t * GPIX:(t + 1) * GPIX],
                        in_=o_sb[:, :])

            # software pipeline: qk(t+1) ahead of rows(t); AV stage one
            # row behind scores so PE always has independent work while
            # ACT computes exp / DVE evacuates
            emit_qk(0)
            from collections import deque
            pend = deque()
            DEPTH = 1
            for t in range(GRP):
                if t + 1 < GRP:
                    emit_qk(t + 1)
                at_tile = persist.tile([128, 4 * GPIX], BF16, tag=f"attn{t}")
                attn_t[t] = at_tile
                for rr in range(4):
                    front = emit_row_front(t, rr)
                    if len(pend) >= DEPTH:
                        emit_row_back(*pend.popleft())
                    pend.append((t, rr) + front)
                if t >= 1:
                    emit_outproj(t - 1)
            while pend:
                emit_row_back(*pend.popleft())
            emit_outproj(GRP - 1)

    if apply_waitfix:
        split_excess_waits(nc)
    return nc


# --- walrus workaround -------------------------------------------------
# The walrus build in this container rejects instructions carrying more
# than a small number of semaphore waits (1 for CTRL-queue NoOp/Drain).
# TileContext's exit drain can exceed that. Split: keep at most one wait
# on the original instruction and insert same-engine NoOps immediately
# before it, each carrying one of the excess waits.
def split_excess_waits(nc):
    import bass_rust
    n_split = 0
    for f in nc.m.functions:
        for blk in f.blocks:
            newlist = []
            changed = False
            for inst in blk.instructions:
                si = inst.sync_info
                w = list(si.on_wait) if si is not None else []
                if len(w) > 1:
                    *pre, last = w
                    for ci, wait in enumerate(pre):
                        nop = mybir.InstNoOp(
                            name=f"{inst.name}-wsplit{ci}", ins=[], outs=[])
                        nop.engine = inst.engine
                        nop.sync_info = bass_rust.SyncInfo(
                            on_update=[], on_wait=[wait])
                        newlist.append(nop)
                    inst.sync_info.on_wait = [last]
                    changed = True
                    n_split += 1
                newlist.append(inst)
            if changed:
                blk.instructions = newlist
    return n_split


def shard_inputs(x, w_qkv, w_out, b_out):
    """Full inputs -> list of 8 per-core input maps."""
    bf = ml_dtypes.bfloat16
    wqk = np.ascontiguousarray(w_qkv[:1024].T).astype(bf).reshape(4, 128, 1024)
    wv = np.ascontiguousarray(w_qkv[1024:].T).astype(bf).reshape(4, 128, 512)
    wo = np.ascontiguousarray(w_out.T).astype(bf).reshape(4, 128, 512)
    bias = b_out.astype(np.float32).reshape(4, 128, 1)
    xb = x.astype(bf)  # [4, 512, 96, 96]
    in_maps = []
    for core in range(NCORES):
        b, half = core // 2, core % 2
        xs = np.ascontiguousarray(
            xb[b, :, half * RPC:(half + 1) * RPC, :]).reshape(4, 128, PIX)
        in_maps.append({"x": xs, "wqk": wqk, "wv": wv, "wo": wo, "bias": bias})
    return in_maps


def unshard_outputs(results):
    out = np.empty((B, C, H, W), np.float32)
    for core in range(NCORES):
        b, half = core // 2, core % 2
        out[core // 2, :, (core % 2) * RPC:((core % 2) + 1) * RPC, :] = (
            results[core]["y"].reshape(C, RPC, W))
    return out


_NC_CACHE = None


def kernel(x, w_qkv, w_out, b_out):
    global _NC_CACHE
    from concourse.bass_utils import run_bass_kernel_spmd
    if _NC_CACHE is None:
        _NC_CACHE = build_nc()
    in_maps = shard_inputs(x, w_qkv, w_out, b_out)
    res = run_bass_kernel_spmd(_NC_CACHE, in_maps, list(range(NCORES)))
    return unshard_outputs(res.results)


# revision 28
# speedup vs baseline: 286.3275x; 1.0035x over previous
"""Trainium2 Bass kernel for AxialAttention (attention along W axis).

Sharding: pure data-parallel over (B=4) x (H split in 2) = 8 shards, one
per NeuronCore. Attention mixes pixels only along W within a single
(b, head, h-row), so splitting H requires no collectives.

Per-core pipeline (shard = [C=512, 48 rows x 96 cols], pixels tiled in
12 groups of 384 = exactly 4 attention rows, so all phases pipeline):
  for each pixel-group t (4 rows):
    1. q/k projection GEMM for group t: w_qk^T-as-lhsT, x-as-rhs
    2. per row r in group: v projection (x-as-lhsT, seq-major out),
       scores^T = k^T.T @ q^T per head (row-group pairs -> separate
       PSUM banks), expS = exp(0.125*scores^T) on ACT (no max
       subtraction: |scores*scale| < 7 for these inputs), AV^T +
       column-sums matmuls, reciprocal + normalize-multiply -> attn_out
    3. out projection GEMM for group t + bias via ACT Identity -> y
"""

import numpy as np
import ml_dtypes

import concourse.bass as bass
import concourse.tile as tile
from concourse import mybir

BF16 = mybir.dt.bfloat16
F32 = mybir.dt.float32

B, C, H, W = 4, 512, 96, 96
HEADS, D = 8, 64
SCALE = 0.125
NCORES = 8
RPC = H // 2          # 48 rows per core
PIX = RPC * W         # 4608 pixels per core
GRP = 12              # pixel groups
GPIX = PIX // GRP     # 384 pixels per group = 4 rows


def build_nc(apply_waitfix=True):
    nc = bass.Bass(trn_type="TRN2")
    x_d = nc.declare_dram_parameter("x", [4, 128, PIX], BF16, isOutput=False)
    wqk_d = nc.declare_dram_parameter("wqk", [4, 128, 1024], BF16, isOutput=False)
    wv_d = nc.declare_dram_parameter("wv", [4, 128, 512], BF16, isOutput=False)
    wo_d = nc.declare_dram_parameter("wo", [4, 128, 512], BF16, isOutput=False)
    bias_d = nc.declare_dram_parameter("bias", [4, 128, 1], F32, isOutput=False)
    y_d = nc.declare_dram_parameter("y", [512, PIX], F32, isOutput=True)

    with tile.TileContext(nc) as tc:
        with (
            tc.tile_pool(name="persist", bufs=1) as persist,
            tc.tile_pool(name="vrow", bufs=6) as vrow,
            tc.tile_pool(name="attn", bufs=6) as attn,
            tc.tile_pool(name="ostage", bufs=4) as ostage,
            tc.tile_pool(name="psA", bufs=4, space="PSUM") as psA,
            tc.tile_pool(name="psB", bufs=2, space="PSUM") as psB,
        ):
            # --- persistent loads (wqk + first x tiles first so the
            # projection GEMMs start as early as possible) --------------
            wqk_t, wv_t, wo_t, bias_t = [], [], [], []
            for cc in range(4):
                wt = persist.tile([128, 1024], BF16, tag=f"wqk{cc}")
                nc.sync.dma_start(out=wt[:, :], in_=wqk_d[cc])
                wqk_t.append(wt)
            # x in growing chunks: early groups land fast, later ones
            # amortize the ~0.65us per-DMA sequencer dispatch cost
            CHUNKS = [(0, 1), (1, 2), (2, 4), (4, 6), (6, 9), (9, 12)]
            x_t = [[None] * GRP for _ in range(4)]
            for ci, (t0, t1) in enumerate(CHUNKS):
                w = (t1 - t0) * GPIX
                for cc in range(4):
                    xt = persist.tile([128, w], BF16, tag=f"x{cc}_c{ci}")
                    dma_eng = nc.gpsimd if ci == 0 else nc.sync
                    dma_eng.dma_start(
                        out=xt[:, :],
                        in_=x_d[cc][:, t0 * GPIX:t1 * GPIX])
                    for t in range(t0, t1):
                        x_t[cc][t] = xt[:, (t - t0) * GPIX:(t - t0 + 1) * GPIX]
                if ci == 0:
                    for cc in range(4):
                        vt = persist.tile([128, 512], BF16, tag=f"wv{cc}")
                        nc.gpsimd.dma_start(out=vt[:, :], in_=wv_d[cc])
                        wv_t.append(vt)
                    ones_t = persist.tile([96, 64], BF16, tag="ones")
                    nc.vector.memset(ones_t[:, :], 1.0)
                elif ci == 1:
                    for cc in range(4):
                        ot = persist.tile([128, 512], BF16, tag=f"wo{cc}")
                        nc.sync.dma_start(out=ot[:, :], in_=wo_d[cc])
                        wo_t.append(ot)
                        bt = persist.tile([128, 1], F32, tag=f"bias{cc}")
                        nc.sync.dma_start(out=bt[:, :], in_=bias_d[cc])
                        bias_t.append(bt)

            qk_t = [[None] * GRP for _ in range(8)]
            attn_t = [None] * GRP

            def emit_qk(t):
                for oc in range(8):
                    qps = psA.tile([128, 512], F32, tag="psA")
                    for cc in range(4):
                        nc.tensor.matmul(
                            qps[:, 0:GPIX],
                            lhsT=wqk_t[cc][:, oc * 128:(oc + 1) * 128],
                            rhs=x_t[cc][t][:, :],
                            start=(cc == 0), stop=(cc == 3),
                        )
                    qt = persist.tile([128, GPIX], BF16, tag=f"qk{oc}_{t}")
                    nc.vector.tensor_copy(out=qt[:, :], in_=qps[:, 0:GPIX])
                    qk_t[oc][t] = qt

            def emit_row_front(t, rr):
                """v projection + scores + exp for row rr of group t."""
                rsl = slice(rr * 96, rr * 96 + 96)
                vps = psA.tile([128, 512], F32, tag="psA")
                for cc in range(4):
                    nc.tensor.matmul(
                        vps[0:96, 0:512],
                        lhsT=x_t[cc][t][:, rsl],
                        rhs=wv_t[cc][:, :],
                        start=(cc == 0), stop=(cc == 3),
                    )
                v_sb = vrow.tile([96, 512], BF16)
                nc.vector.tensor_copy(out=v_sb[:, :], in_=vps[0:96, 0:512])

                # scores^T per head: [j, i]; concurrent row-group
                # (K-offset 0 vs 64) matmuls must hit different PSUM
                # banks: head h -> bank h%2
                sps = psB.tile([128, 1024], F32, tag="psB")
                for h in range(8):
                    qc, half = h // 2, 64 * (h % 2)
                    col = 512 * (h % 2) + 96 * (h // 2)
                    nc.tensor.matmul(
                        sps[0:96, col:col + 96],
                        lhsT=qk_t[4 + qc][t][half:half + 64, rsl],
                        rhs=qk_t[qc][t][half:half + 64, rsl],
                    )
                expS = attn.tile([96, 768], BF16)
                for k in range(2):
                    nc.scalar.activation(
                        out=expS[:, k * 384:(k + 1) * 384],
                        in_=sps[0:96, k * 512:k * 512 + 384],
                        func=mybir.ActivationFunctionType.Exp,
                        scale=SCALE,
                    )
                return v_sb, expS

            def emit_row_back(t, rr, v_sb, expS):
                """AV + sums matmuls, reciprocal, normalize for a row."""
                rsl = slice(rr * 96, rr * 96 + 96)
                at_c = attn_t[t].rearrange("p (c n) -> p c n", c=4)
                aps = psB.tile([128, 1024], F32, tag="psB")
                for h in range(8):
                    half, blk = 64 * (h % 2), 96 * (h // 2)
                    ecol = 384 * (h % 2) + 96 * (h // 2)
                    nc.tensor.matmul(
                        aps[half:half + 64, blk:blk + 96],
                        lhsT=v_sb[:, h * 64:(h + 1) * 64],
                        rhs=expS[:, ecol:ecol + 96],
                    )
                # column sums for all heads of one parity in one matmul
                # (expS is parity-major: cols 0:384 = even heads)
                for par in range(2):
                    nc.tensor.matmul(
                        aps[64 * par:64 * par + 64, 512:896],
                        lhsT=ones_t[:, :],
                        rhs=expS[:, 384 * par:384 * par + 384],
                    )
                recip = attn.tile([128, 384], F32)
                nc.vector.reciprocal(out=recip[:, :], in_=aps[:, 512:896])
                nc.vector.tensor_tensor(
                    out=at_c[:, :, rsl],
                    in0=aps[:, 0:384].rearrange("p (c i) -> p c i", c=4),
                    in1=recip.rearrange("p (c i) -> p c i", c=4),
                    op=mybir.AluOpType.mult,
                )

            def emit_outproj(t, ocs=range(4)):
                at_c = attn_t[t].rearrange("p (c n) -> p c n", c=4)
                for oc in ocs:
                    ops_ = psA.tile([128, 512], F32, tag="psA")
                    for cc in range(4):
                        nc.tensor.matmul(
                            ops_[:, 0:GPIX],
                            lhsT=wo_t[cc][:, oc * 128:(oc + 1) * 128],
                            rhs=at_c[:, cc, :],
                            start=(cc == 0), stop=(cc == 3),
                        )
                    o_sb = ostage.tile([128, GPIX], F32)
                    nc.scalar.add(out=o_sb[:, :], in_=ops_[:, 0:GPIX],
                                  add=bias_t[oc][:, :])
                    nc.gpsimd.dma_start(
                        out=y_d[oc * 128:(oc + 1) * 128,
                                t * GPIX:(t + 1) * GPIX],
                        in_=o_sb[:, :])

            # software pipeline: qk(t+1) ahead of rows(t); AV stage one
            # row behind scores so PE always has independent work while
            # ACT computes exp / DVE evacuates
            emit_qk(0)
            from collections import deque
            pend = deque()
            DEPTH = 1
            for t in range(GRP):
                if t + 1 < GRP:
                    emit_qk(t + 1)
                at_tile = persist.tile([128, 4 * GPIX], BF16, tag=f"attn{t}")
                attn_t[t] = at_tile
                for rr in range(4):
                    front = emit_row_front(t, rr)
                    if len(pend) >= DEPTH:
                        emit_row_back(*pend.popleft())
                    pend.append((t, rr) + front)
                if t >= 1:
                    emit_outproj(t - 1)
            while pend:
                emit_row_back(*pend.popleft())
            emit_outproj(GRP - 1)

    if apply_waitfix:
        split_excess_waits(nc)
    return nc


# --- walrus workaround -------------------------------------------------
# The walrus build in this container rejects instructions carrying more
# than a small number of semaphore waits (1 for CTRL-queue NoOp/Drain).
# TileContext's exit drain can exceed that. Split: keep at most one wait
# on the original instruction and insert same-engine NoOps immediately
# before it, each carrying one of the excess waits.
def split_excess_waits(nc):
    import bass_rust
    n_split = 0
    for f in nc.m.functions:
        for blk in f.blocks:
            newlist = []
            changed = False
            for inst in blk.instructions:
                si = inst.sync_info
                w = list(si.on_wait) if si is not None else []
                if len(w) > 1:
                    *pre, last = w
                    for ci, wait in enumerate(pre):
                        nop = mybir.InstNoOp(
                            name=f"{inst.name}-wsplit{ci}", ins=[], outs=[])
                        nop.engine = inst.engine
                        nop.sync_info = bass_rust.SyncInfo(
                            on_update=[], on_wait=[wait])
                        newlist.append(nop)
                    inst.sync_info.on_wait = [last]
                    changed = True
                    n_split += 1
                newlist.append(inst)
            if changed:
                blk.instructions = newlist
    return n_split


def shard_inputs(x, w_qkv, w_out, b_out):
    """Full inputs -> list of 8 per-core input maps."""
    bf = ml_dtypes.bfloat16
    wqk = np.ascontiguousarray(w_qkv[:1024].T).astype(bf).reshape(4, 128, 1024)
    wv = np.ascontiguousarray(w_qkv[1024:].T).astype(bf).reshape(4, 128, 512)
    wo = np.ascontiguousarray(w_out.T).astype(bf).reshape(4, 128, 512)
    bias = b_out.astype(np.float32).reshape(4, 128, 1)
    xb = x.astype(bf)  # [4, 512, 96, 96]
    in_maps = []
    for core in range(NCORES):
        b, half = core // 2, core % 2
        xs = np.ascontiguousarray(
            xb[b, :, half * RPC:(half + 1) * RPC, :]).reshape(4, 128, PIX)
        in_maps.append({"x": xs, "wqk": wqk, "wv": wv, "wo": wo, "bias": bias})
    return in_maps


def unshard_outputs(results):
    out = np.empty((B, C, H, W), np.float32)
    for core in range(NCORES):
        b, half = core // 2, core % 2
        out[core // 2, :, (core % 2) * RPC:((core % 2) + 1) * RPC, :] = (
            results[core]["y"].reshape(C, RPC, W))
    return out


_NC_CACHE = None


def kernel(x, w_qkv, w_out, b_out):
    global _NC_CACHE
    from concourse.bass_utils import run_bass_kernel_spmd
    if _NC_CACHE is None:
        _NC_CACHE = build_nc()
    in_maps = shard_inputs(x, w_qkv, w_out, b_out)
    res = run_bass_kernel_spmd(_NC_CACHE, in_maps, list(range(NCORES)))
    return unshard_outputs(res.results)
